# revision 2
# baseline (speedup 1.0000x reference)
"""Varlen causal GQA attention (B=4, S=1024, HQ=32, HK=8, D=128, fp32)
on 8 Trainium2 NeuronCores.

Sharding: tensor-parallel over the 8 kv heads (GQA groups stay together):
core i gets kv head i and query heads [4i, 4i+4), all 4 sequences. No
collectives; gather = concat along the head axis on host.

Per-core kernel (per seq b, head h, 512-query chunk qc):
  scores_T[k,q] = K_tile^T.T @ Q^T       (float32r matmul, full PE rate)
  P_T = exp(scale * scores_T)            (ScalarE, PSUM->SBUF fp16)
  P_T *= causal_mask (diagonal blocks)   (DVE)
  O[q,:128|128] += P_T_slice.T @ [V|1]   (fp16 matmul; col 128 = softmax sum)
  O = O[:, :128] * 1/O[:, 128]           (DVE reciprocal + scalar mul)
Q/K arrive host-pre-transposed to (d, token) layout; V as fp16.
"""

import numpy as np
import ml_dtypes

import concourse.bass as bass
import concourse.tile as tile
import concourse.mybir as mybir
from concourse import bacc
from concourse.bass_utils import run_bass_kernel_spmd

B, S, D = 4, 1024, 128
HQ, HK = 32, 8
G = HQ // HK          # query heads per kv head (= per core)
N_CORES = 8
SCALE = 1.0 / float(np.sqrt(D))
QCW = 512             # query-chunk width (matmul moving free dim)
QC = S // QCW         # query chunks per sequence
KTW = 128             # key-tile width (matmul stationary free dim)
KT = S // KTW         # key tiles per sequence
QI = QCW // 128       # 128-query subtiles per chunk

F32 = mybir.dt.float32
F32R = mybir.dt.float32r
FP16 = mybir.dt.float16


def build_nc(repeat: int = 1):
    """Build the single-core Bass program (SPMD across 8 cores).

    repeat > 1 wraps the body in a hardware loop — used only for timing
    (marginal wall time per iteration approximates HW kernel time).
    """
    nc = bacc.Bacc(None, target_bir_lowering=False, debug=False)

    qT = nc.dram_tensor("qT", [G, B, D, S], F32R, kind="ExternalInput")
    kT = nc.dram_tensor("kT", [B, D, S], F32R, kind="ExternalInput")
    v = nc.dram_tensor("v", [B, S, D], FP16, kind="ExternalInput")
    mk = nc.dram_tensor("mk", [D, QI * QCW], FP16, kind="ExternalInput")
    o = nc.dram_tensor("o", [B * S, G, D], F32, kind="ExternalOutput")
    # (b, qc, g, p, qi, d) view of the output for per-chunk stores
    o_r = o[:].rearrange("(b qc qi p) g d -> b qc g p qi d", b=B, qc=QC, qi=QI, p=128)

    with tile.TileContext(nc) as tc:
        with (
            tc.tile_pool(name="cpool", bufs=1) as cpool,
            tc.tile_pool(name="kpool", bufs=2) as kpool,
            tc.tile_pool(name="vpool", bufs=2) as vpool,
            tc.tile_pool(name="qpool", bufs=2) as qpool,
            tc.tile_pool(name="ppool", bufs=22) as ppool,
            tc.tile_pool(name="opool", bufs=4) as opool,
            tc.tile_pool(name="rpool", bufs=6) as rpool,
            tc.tile_pool(name="ps_s", bufs=3, space="PSUM") as ps_s,
            tc.tile_pool(name="ps_o", bufs=4, space="PSUM") as ps_o,
        ):
            mask_t = cpool.tile([128, QI * QCW], FP16)
            nc.sync.dma_start(out=mask_t[:], in_=mk[:])

            def emit_pv(st):
                """Probs @ [V|1] for one (b, h, qc) chunk, then normalize + store."""
                b, h, qc, p_tiles, v_t = st
                for qih in range(QI // 2):
                    # two 128-query accumulation chains share one PSUM bank
                    po = ps_o.tile([128, 2, KTW + 1], F32, tag="po", name="po")
                    for q2 in range(2):
                        qi = qih * 2 + q2
                        qg = qc * QI + qi      # global 128-query block index
                        for kt in range(qg + 1):
                            nc.tensor.matmul(
                                po[:, q2, :],
                                lhsT=p_tiles[kt][:, qi * 128:(qi + 1) * 128],
                                rhs=v_t[:, kt, :],
                                start=(kt == 0),
                                stop=(kt == qg),
                            )
                    rec = rpool.tile([128, 2], F32, tag="rec", name="rec")
                    nc.vector.reciprocal(rec[:], po[:, :, KTW])
                    ot = opool.tile([128, 2, KTW], F32, tag="ot", name="ot")
                    for q2 in range(2):
                        nc.vector.tensor_scalar_mul(
                            ot[:, q2, :], po[:, q2, 0:KTW], rec[:, q2:q2 + 1]
                        )
                    nc.sync.dma_start(
                        out=o_r[b, qc, h][:, qih * 2:qih * 2 + 2, :], in_=ot[:]
                    )

            def body(_iv=None):
                pending = None  # one-chunk-deep software pipeline
                for b in range(B):
                    kt_t = kpool.tile([128, S], F32R, tag="kt", name="kt_t")
                    nc.sync.dma_start(out=kt_t[:], in_=kT[b])
                    v_t = vpool.tile([128, KT, KTW + 1], FP16, tag="vt", name="v_t")
                    nc.sync.dma_start(
                        out=v_t[:, :, 0:KTW],
                        in_=v[b].rearrange("(kt p) d -> p kt d", p=128),
                    )
                    nc.vector.memset(v_t[:, :, KTW:KTW + 1], 1.0)
                    for h in range(G):
                        q_t = qpool.tile([128, S], F32R, tag="qt", name="q_t")
                        nc.sync.dma_start(out=q_t[:], in_=qT[h, b])
                        for qc in range(QC):
                            p_tiles = []
                            for kt in range(QI * (qc + 1)):
                                j = kt - QI * qc  # >= 0: diagonal (masked) block
                                c0 = 128 * j if j > 0 else 0
                                ps = ps_s.tile([128, QCW], F32, tag="ps", name="ps")
                                nc.tensor.matmul(
                                    ps[:, c0:QCW],
                                    lhsT=kt_t[:, kt * KTW:(kt + 1) * KTW],
                                    rhs=q_t[:, qc * QCW + c0:(qc + 1) * QCW],
                                    start=True,
                                    stop=True,
                                )
                                pt = ppool.tile([128, QCW], FP16, tag="pt", name="pt")
                                nc.scalar.activation(
                                    pt[:, c0:QCW],
                                    ps[:, c0:QCW],
                                    mybir.ActivationFunctionType.Exp,
                                    scale=SCALE,
                                )
                                if j >= 0:
                                    nc.vector.tensor_mul(
                                        pt[:, c0:QCW],
                                        pt[:, c0:QCW],
                                        mask_t[:, j * QCW + c0:(j + 1) * QCW],
                                    )
                                p_tiles.append(pt)
                            if pending is not None:
                                emit_pv(pending)
                            pending = (b, h, qc, p_tiles, v_t)
                if pending is not None:
                    emit_pv(pending)

            if repeat == 1:
                body()
            else:
                with tc.For_i(0, repeat, 1) as iv:
                    body(iv)

    nc.compile()
    return nc


def _build_mask() -> np.ndarray:
    """mask[kk, j*QCW + qq] = 1 if query qq attends key kk of diagonal block j."""
    kk = np.arange(128)[:, None]
    qq = np.arange(QCW)[None, :]
    cols = [(qq >= kk + 128 * j).astype(np.float32) for j in range(QI)]
    return np.concatenate(cols, axis=1).astype(np.float16)


def _core_inputs(q: np.ndarray, k: np.ndarray, v: np.ndarray):
    """Slice + lay out per-core inputs. Host-side shard/layout step."""
    mask = _build_mask()
    q5 = q.reshape(B, S, HK, G, D)
    k4 = k.reshape(B, S, HK, D)
    v4 = v.reshape(B, S, HK, D)
    in_maps = []
    for c in range(N_CORES):
        qT = np.ascontiguousarray(q5[:, :, c, :, :].transpose(2, 0, 3, 1))  # (G,B,D,S)
        kT = np.ascontiguousarray(k4[:, :, c, :].transpose(0, 2, 1))        # (B,D,S)
        vb = np.ascontiguousarray(v4[:, :, c, :]).astype(np.float16)
        in_maps.append({"qT": qT, "kT": kT, "v": vb, "mk": mask})
    return in_maps


_NC_CACHE = {}


def kernel(q, k, v, cu_seqlens_q=None, cu_seqlens_k=None,
           max_seqlen_q=None, max_seqlen_k=None) -> np.ndarray:
    q = np.asarray(q, dtype=np.float32)
    k = np.asarray(k, dtype=np.float32)
    v = np.asarray(v, dtype=np.float32)
    assert q.shape == (B * S, HQ, D) and k.shape == (B * S, HK, D)

    if "nc" not in _NC_CACHE:
        _NC_CACHE["nc"] = build_nc(repeat=1)
    nc = _NC_CACHE["nc"]

    in_maps = _core_inputs(q, k, v)
    res = run_bass_kernel_spmd(nc, in_maps, core_ids=list(range(N_CORES)))

    out = np.empty((B * S, HQ, D), np.float32)
    for c in range(N_CORES):
        out[:, c * G:(c + 1) * G, :] = res.results[c]["o"]
    return out


# revision 3
# speedup vs baseline: 2.2666x; 2.2666x over previous
"""Varlen causal GQA attention (B=4, S=1024, HQ=32, HK=8, D=128, fp32)
on 8 Trainium2 NeuronCores.

Sharding: tensor-parallel over the 8 kv heads (GQA groups stay together):
core i gets kv head i and query heads [4i, 4i+4), all 4 sequences. No
collectives; gather = concat along the head axis on host.

Per-core kernel (per seq b, head h, 512-query chunk qc):
  scores_T[k,q] = K_tile^T.T @ Q^T       (float32r matmul, full PE rate)
  P_T = exp(scale * scores_T)            (ScalarE, PSUM->SBUF fp16)
  P_T *= causal triangle (diag 128 cols) (DVE, shared (128,128) mask)
  O[q,:128|128] += P_T_slice.T @ [V|1]   (fp16 matmul; col 128 = softmax sum)
  O = O[:, :128] * 1/O[:, 128]           (DVE reciprocal + scalar mul)
Q/K arrive host-pre-transposed to (d, token) layout; V as fp16.
"""

import numpy as np
import ml_dtypes

import concourse.bass as bass
import concourse.tile as tile
import concourse.mybir as mybir
from concourse import bacc
from concourse.bass_utils import run_bass_kernel_spmd

B, S, D = 4, 1024, 128
HQ, HK = 32, 8
G = HQ // HK          # query heads per kv head (= per core)
N_CORES = 8
SCALE = 1.0 / float(np.sqrt(D))
QCW = 512             # query-chunk width (matmul moving free dim)
QC = S // QCW         # query chunks per sequence
KTW = 128             # key-tile width (matmul stationary free dim)
KT = S // KTW         # key tiles per sequence
QI = QCW // 128       # 128-query subtiles per chunk

F32 = mybir.dt.float32
F32R = mybir.dt.float32r
FP16 = mybir.dt.float16


def build_nc(repeat: int = 1, qk_dtype=F32R):
    """Build the single-core Bass program (SPMD across 8 cores).

    repeat > 1 wraps the body in a hardware loop — used only for timing
    (marginal wall time per iteration approximates HW kernel time).
    """
    nc = bacc.Bacc(None, target_bir_lowering=False, debug=False)

    qT = nc.dram_tensor("qT", [G, B, D, S], qk_dtype, kind="ExternalInput")
    kT = nc.dram_tensor("kT", [B, D, S], qk_dtype, kind="ExternalInput")
    v = nc.dram_tensor("v", [B, S, D], FP16, kind="ExternalInput")
    mk = nc.dram_tensor("mk", [D, KTW], FP16, kind="ExternalInput")
    o = nc.dram_tensor("o", [B * S, G, D], F32, kind="ExternalOutput")
    # (b, g, p, qi8, d) view of the output for per-(b,h) stores
    o_r = o[:].rearrange("(b qi p) g d -> b g p qi d", b=B, qi=S // 128, p=128)

    with tile.TileContext(nc) as tc:
        with (
            tc.tile_pool(name="cpool", bufs=1) as cpool,
            tc.tile_pool(name="kpool", bufs=2) as kpool,
            tc.tile_pool(name="vpool", bufs=2) as vpool,
            tc.tile_pool(name="qpool", bufs=2) as qpool,
            tc.tile_pool(name="ppool", bufs=11) as ppool,
            tc.tile_pool(name="p2pool", bufs=4) as p2pool,
            tc.tile_pool(name="opool", bufs=3) as opool,
            tc.tile_pool(name="rpool", bufs=6) as rpool,
            tc.tile_pool(name="ps_s", bufs=2, space="PSUM") as ps_s,
            tc.tile_pool(name="ps_s2", bufs=2, space="PSUM") as ps_s2,
            tc.tile_pool(name="ps_o", bufs=2, space="PSUM") as ps_o,
        ):
            # shared causal triangle: mask[kk, q] = 1 iff q >= kk
            mask_t = cpool.tile([128, KTW], FP16)
            nc.sync.dma_start(out=mask_t[:], in_=mk[:])

            def emit_scores(b, h, qc, q_t, kt_t):
                """QK^T + exp (+ triangle mask on the 128 partial cols of
                diagonal blocks). Returns [(tile, col_off)] per key tile."""
                p_tiles = []
                # full (non-diagonal) blocks, two per 2-bank psum tile,
                # one exp per 1024 columns
                nfull = QI * qc
                for kp in range(nfull // 2):
                    ps2 = ps_s2.tile([128, 2, QCW], F32, tag="ps2", name="ps2")
                    for i in range(2):
                        kt = kp * 2 + i
                        nc.tensor.matmul(
                            ps2[:, i, :],
                            lhsT=kt_t[:, kt * KTW:(kt + 1) * KTW],
                            rhs=q_t[:, qc * QCW:(qc + 1) * QCW],
                            start=True, stop=True,
                        )
                    pt2 = p2pool.tile([128, 2 * QCW], FP16, tag="pt2", name="pt2")
                    nc.scalar.activation(
                        pt2[:].rearrange("p (i q) -> p i q", i=2), ps2[:],
                        mybir.ActivationFunctionType.Exp, scale=SCALE,
                    )
                    p_tiles.append((pt2, 0))
                    p_tiles.append((pt2, QCW))
                # diagonal blocks: only cols >= c0 are live; mask-mul only
                # the 128 partial cols [c0, c0+128)
                for j in range(QI):
                    kt = nfull + j
                    c0 = 128 * j
                    ps = ps_s.tile([128, QCW], F32, tag="ps", name="ps")
                    nc.tensor.matmul(
                        ps[:, c0:QCW],
                        lhsT=kt_t[:, kt * KTW:(kt + 1) * KTW],
                        rhs=q_t[:, qc * QCW + c0:(qc + 1) * QCW],
                        start=True, stop=True,
                    )
                    pt = ppool.tile([128, QCW], FP16, tag="pt", name="pt")
                    nc.scalar.activation(
                        pt[:, c0:QCW], ps[:, c0:QCW],
                        mybir.ActivationFunctionType.Exp, scale=SCALE,
                    )
                    nc.vector.tensor_mul(
                        pt[:, c0:c0 + 128], pt[:, c0:c0 + 128], mask_t[:]
                    )
                    p_tiles.append((pt, 0))
                return p_tiles

            def emit_pv(st):
                """Probs @ [V|1] for one (b, h, qc) chunk, then normalize."""
                b, h, qc, p_tiles, v_t, o_t = st
                for qih in range(QI // 2):
                    # two 128-query accumulation chains share one PSUM bank
                    po = ps_o.tile([128, 2, KTW + 1], F32, tag="po", name="po")
                    for q2 in range(2):
                        qi = qih * 2 + q2
                        qg = qc * QI + qi      # global 128-query block index
                        for kt in range(qg + 1):
                            pt, off = p_tiles[kt]
                            nc.tensor.matmul(
                                po[:, q2, :],
                                lhsT=pt[:, off + qi * 128:off + (qi + 1) * 128],
                                rhs=v_t[:, kt, :],
                                start=(kt == 0),
                                stop=(kt == qg),
                            )
                    rec = rpool.tile([128, 2], F32, tag="rec", name="rec")
                    nc.vector.reciprocal(rec[:], po[:, :, KTW])
                    for q2 in range(2):
                        nc.vector.tensor_scalar_mul(
                            o_t[:, qc * QI + qih * 2 + q2, :],
                            po[:, q2, 0:KTW], rec[:, q2:q2 + 1],
                        )
                if qc == QC - 1:
                    nc.gpsimd.dma_start(out=o_r[b, h], in_=o_t[:])

            def body(_iv=None):
                pending = None  # one-chunk-deep software pipeline
                for b in range(B):
                    kt_t = kpool.tile([128, S], qk_dtype, tag="kt", name="kt_t")
                    nc.sync.dma_start(out=kt_t[:], in_=kT[b])
                    v_t = vpool.tile([128, KT, KTW + 1], FP16, tag="vt", name="v_t")
                    nc.sync.dma_start(
                        out=v_t[:, :, 0:KTW],
                        in_=v[b].rearrange("(kt p) d -> p kt d", p=128),
                    )
                    nc.vector.memset(v_t[:, :, KTW:KTW + 1], 1.0)
                    for h in range(G):
                        q_t = qpool.tile([128, S], qk_dtype, tag="qt", name="q_t")
                        nc.sync.dma_start(out=q_t[:], in_=qT[h, b])
                        o_t = opool.tile([128, S // 128, KTW], F32, tag="ot",
                                         name="o_t")
                        for qc in range(QC):
                            p_tiles = emit_scores(b, h, qc, q_t, kt_t)
                            if pending is not None:
                                emit_pv(pending)
                            pending = (b, h, qc, p_tiles, v_t, o_t)
                if pending is not None:
                    emit_pv(pending)

            if repeat == 1:
                body()
            else:
                with tc.For_i(0, repeat, 1) as iv:
                    body(iv)

    nc.compile()
    return nc


def _build_mask() -> np.ndarray:
    """Shared diagonal-block triangle: mask[kk, q] = 1 iff q >= kk."""
    kk = np.arange(128)[:, None]
    qq = np.arange(KTW)[None, :]
    return (qq >= kk).astype(np.float16)


def _core_inputs(q: np.ndarray, k: np.ndarray, v: np.ndarray,
                 qk_np=np.float32):
    """Slice + lay out per-core inputs. Host-side shard/layout step."""
    mask = _build_mask()
    q5 = q.reshape(B, S, HK, G, D)
    k4 = k.reshape(B, S, HK, D)
    v4 = v.reshape(B, S, HK, D)
    in_maps = []
    for c in range(N_CORES):
        qT = np.ascontiguousarray(
            q5[:, :, c, :, :].transpose(2, 0, 3, 1)).astype(qk_np)  # (G,B,D,S)
        kT = np.ascontiguousarray(
            k4[:, :, c, :].transpose(0, 2, 1)).astype(qk_np)        # (B,D,S)
        vb = np.ascontiguousarray(v4[:, :, c, :]).astype(np.float16)
        in_maps.append({"qT": qT, "kT": kT, "v": vb, "mk": mask})
    return in_maps


_NC_CACHE = {}


def kernel(q, k, v, cu_seqlens_q=None, cu_seqlens_k=None,
           max_seqlen_q=None, max_seqlen_k=None) -> np.ndarray:
    q = np.asarray(q, dtype=np.float32)
    k = np.asarray(k, dtype=np.float32)
    v = np.asarray(v, dtype=np.float32)
    assert q.shape == (B * S, HQ, D) and k.shape == (B * S, HK, D)

    if "nc" not in _NC_CACHE:
        _NC_CACHE["nc"] = build_nc(repeat=1)
    nc = _NC_CACHE["nc"]

    in_maps = _core_inputs(q, k, v)
    res = run_bass_kernel_spmd(nc, in_maps, core_ids=list(range(N_CORES)))

    out = np.empty((B * S, HQ, D), np.float32)
    for c in range(N_CORES):
        out[:, c * G:(c + 1) * G, :] = res.results[c]["o"]
    return out


# revision 4
# speedup vs baseline: 2.3655x; 1.0436x over previous
"""Varlen causal GQA attention (B=4, S=1024, HQ=32, HK=8, D=128, fp32)
on 8 Trainium2 NeuronCores.

Sharding: tensor-parallel over the 8 kv heads (GQA groups stay together):
core i gets kv head i and query heads [4i, 4i+4), all 4 sequences. No
collectives; gather = concat along the head axis on host.

Per-core kernel, per (seq b, head h) with the full 1024-query chunk:
  for each 128-key tile kt, over live query cols [128*kt, 1024):
    scores_T[k,q] = K_tile^T.T @ Q^T     (float32r matmul, <=512-col pieces)
    P_T = exp(scale * scores_T)          (ScalarE, PSUM->SBUF fp16)
    P_T[:, :128] *= causal triangle      (DVE, shared (128,128) mask)
  for each 128-query block qi (two PSUM chains per bank):
    O[q,:128|128] += P_T_slice.T @ [V|1] (fp16 matmul; col 128 = sum exp)
    O = O[:, :128] * 1/O[:, 128]         (DVE reciprocal + scalar mul)
Q/K arrive host-pre-transposed to (d, token) layout; V as fp16.
"""

import numpy as np
import ml_dtypes

import concourse.bass as bass
import concourse.tile as tile
import concourse.mybir as mybir
from concourse import bacc
from concourse.bass_utils import run_bass_kernel_spmd

B, S, D = 4, 1024, 128
HQ, HK = 32, 8
G = HQ // HK          # query heads per kv head (= per core)
N_CORES = 8
SCALE = 1.0 / float(np.sqrt(D))
KTW = 128             # key-tile width (matmul stationary free dim)
KT = S // KTW         # key tiles per sequence
NQI = S // 128        # 128-query blocks per sequence
MMW = 512             # max matmul moving free dim

F32 = mybir.dt.float32
F32R = mybir.dt.float32r
FP16 = mybir.dt.float16


def build_nc(repeat: int = 1, qk_dtype=F32R):
    """Build the single-core Bass program (SPMD across 8 cores).

    repeat > 1 wraps the body in a hardware loop — used only for timing
    (marginal wall time per iteration approximates HW kernel time).
    """
    nc = bacc.Bacc(None, target_bir_lowering=False, debug=False)

    qT = nc.dram_tensor("qT", [G, B, D, S], qk_dtype, kind="ExternalInput")
    kT = nc.dram_tensor("kT", [B, D, S], qk_dtype, kind="ExternalInput")
    v = nc.dram_tensor("v", [B, S, D], FP16, kind="ExternalInput")
    mk = nc.dram_tensor("mk", [D, KTW], FP16, kind="ExternalInput")
    o = nc.dram_tensor("o", [B * S, G, D], F32, kind="ExternalOutput")
    # (b, g, p, qi, d) view of the output for per-(b,h) stores
    o_r = o[:].rearrange("(b qi p) g d -> b g p qi d", b=B, qi=NQI, p=128)

    with tile.TileContext(nc) as tc:
        with (
            tc.tile_pool(name="cpool", bufs=1) as cpool,
            tc.tile_pool(name="kpool", bufs=2) as kpool,
            tc.tile_pool(name="vpool", bufs=2) as vpool,
            tc.tile_pool(name="qpool", bufs=2) as qpool,
            tc.tile_pool(name="ppool", bufs=18) as ppool,
            tc.tile_pool(name="opool", bufs=3) as opool,
            tc.tile_pool(name="rpool", bufs=8) as rpool,
            tc.tile_pool(name="ps2", bufs=2, space="PSUM") as ps2,
            tc.tile_pool(name="ps1", bufs=2, space="PSUM") as ps1,
            tc.tile_pool(name="ps_o", bufs=2, space="PSUM") as ps_o,
        ):
            # shared causal triangle: mask[kk, q] = 1 iff q >= kk
            mask_t = cpool.tile([128, KTW], FP16)
            nc.sync.dma_start(out=mask_t[:], in_=mk[:])

            def emit_scores(b, h, q_t, kt_t):
                """QK^T + exp + triangle mask for all 8 key tiles.

                Key tile kt's live query range is [128*kt, S); its P_T tile
                stores those cols at local offset 0."""
                p_tiles = []
                for kt in range(KT):
                    c0 = KTW * kt           # first live query col (global)
                    w = S - c0              # live width
                    lhsT = kt_t[:, kt * KTW:(kt + 1) * KTW]
                    if c0 < MMW:
                        ps = ps2.tile([128, 2 * MMW], F32, tag="ps2", name="ps")
                        lo = c0
                        nc.tensor.matmul(
                            ps[:, c0:MMW], lhsT=lhsT, rhs=q_t[:, c0:MMW],
                            start=True, stop=True,
                        )
                        nc.tensor.matmul(
                            ps[:, MMW:S], lhsT=lhsT, rhs=q_t[:, MMW:S],
                            start=True, stop=True,
                        )
                    else:
                        ps = ps1.tile([128, MMW], F32, tag="ps1", name="ps")
                        lo = c0 - MMW
                        nc.tensor.matmul(
                            ps[:, lo:MMW], lhsT=lhsT, rhs=q_t[:, c0:S],
                            start=True, stop=True,
                        )
                    pt = ppool.tile([128, S], FP16, tag="pt", name="pt")
                    nc.scalar.activation(
                        pt[:, 0:w], ps[:, lo:lo + w],
                        mybir.ActivationFunctionType.Exp, scale=SCALE,
                    )
                    nc.vector.tensor_mul(pt[:, 0:KTW], pt[:, 0:KTW], mask_t[:])
                    p_tiles.append(pt)
                return p_tiles

            def emit_pv(st):
                """Probs @ [V|1] for one (b, h), then normalize + store."""
                b, h, p_tiles, v_t, o_t = st
                for qih in range(NQI // 2):
                    # two 128-query accumulation chains share one PSUM bank
                    po = ps_o.tile([128, 2, KTW + 1], F32, tag="po", name="po")
                    for q2 in range(2):
                        qi = qih * 2 + q2
                        for kt in range(qi + 1):
                            nc.tensor.matmul(
                                po[:, q2, :],
                                lhsT=p_tiles[kt][:, (qi - kt) * 128:
                                                 (qi - kt + 1) * 128],
                                rhs=v_t[:, kt, :],
                                start=(kt == 0),
                                stop=(kt == qi),
                            )
                    rec = rpool.tile([128, 2], F32, tag="rec", name="rec")
                    nc.vector.reciprocal(rec[:], po[:, :, KTW])
                    for q2 in range(2):
                        nc.vector.tensor_scalar_mul(
                            o_t[:, qih * 2 + q2, :],
                            po[:, q2, 0:KTW], rec[:, q2:q2 + 1],
                        )
                nc.gpsimd.dma_start(out=o_r[b, h], in_=o_t[:])

            def body(_iv=None):
                pending = None  # one-(b,h)-deep software pipeline
                for b in range(B):
                    kt_t = kpool.tile([128, S], qk_dtype, tag="kt", name="kt_t")
                    nc.sync.dma_start(out=kt_t[:], in_=kT[b])
                    v_t = vpool.tile([128, KT, KTW + 1], FP16, tag="vt", name="v_t")
                    nc.sync.dma_start(
                        out=v_t[:, :, 0:KTW],
                        in_=v[b].rearrange("(kt p) d -> p kt d", p=128),
                    )
                    nc.vector.memset(v_t[:, :, KTW:KTW + 1], 1.0)
                    for h in range(G):
                        q_t = qpool.tile([128, S], qk_dtype, tag="qt", name="q_t")
                        nc.sync.dma_start(out=q_t[:], in_=qT[h, b])
                        o_t = opool.tile([128, NQI, KTW], F32, tag="ot",
                                         name="o_t")
                        p_tiles = emit_scores(b, h, q_t, kt_t)
                        if pending is not None:
                            emit_pv(pending)
                        pending = (b, h, p_tiles, v_t, o_t)
                if pending is not None:
                    emit_pv(pending)

            if repeat == 1:
                body()
            else:
                with tc.For_i(0, repeat, 1) as iv:
                    body(iv)

    nc.compile()
    return nc


def _build_mask() -> np.ndarray:
    """Shared diagonal-block triangle: mask[kk, q] = 1 iff q >= kk."""
    kk = np.arange(128)[:, None]
    qq = np.arange(KTW)[None, :]
    return (qq >= kk).astype(np.float16)


def _core_inputs(q: np.ndarray, k: np.ndarray, v: np.ndarray,
                 qk_np=np.float32):
    """Slice + lay out per-core inputs. Host-side shard/layout step."""
    mask = _build_mask()
    q5 = q.reshape(B, S, HK, G, D)
    k4 = k.reshape(B, S, HK, D)
    v4 = v.reshape(B, S, HK, D)
    in_maps = []
    for c in range(N_CORES):
        qT = np.ascontiguousarray(
            q5[:, :, c, :, :].transpose(2, 0, 3, 1)).astype(qk_np)  # (G,B,D,S)
        kT = np.ascontiguousarray(
            k4[:, :, c, :].transpose(0, 2, 1)).astype(qk_np)        # (B,D,S)
        vb = np.ascontiguousarray(v4[:, :, c, :]).astype(np.float16)
        in_maps.append({"qT": qT, "kT": kT, "v": vb, "mk": mask})
    return in_maps


_NC_CACHE = {}


def kernel(q, k, v, cu_seqlens_q=None, cu_seqlens_k=None,
           max_seqlen_q=None, max_seqlen_k=None) -> np.ndarray:
    q = np.asarray(q, dtype=np.float32)
    k = np.asarray(k, dtype=np.float32)
    v = np.asarray(v, dtype=np.float32)
    assert q.shape == (B * S, HQ, D) and k.shape == (B * S, HK, D)

    if "nc" not in _NC_CACHE:
        _NC_CACHE["nc"] = build_nc(repeat=1)
    nc = _NC_CACHE["nc"]

    in_maps = _core_inputs(q, k, v)
    res = run_bass_kernel_spmd(nc, in_maps, core_ids=list(range(N_CORES)))

    out = np.empty((B * S, HQ, D), np.float32)
    for c in range(N_CORES):
        out[:, c * G:(c + 1) * G, :] = res.results[c]["o"]
    return out
